# revision 6
# baseline (speedup 1.0000x reference)
"""Bass/Trainium2 kernel for BailingAttention (GQA prefill, causal, RoPE).

Sharding: tensor-parallel over heads across 8 NeuronCores. Each core computes
2 query heads + its group's shared KV head end-to-end (QKV projection, RoPE,
causal attention, output projection) and writes a partial [T, HID] output;
the host sums the 8 partials (the row-parallel all-reduce).

Layouts on device (partition dim first):
  hiddenT  [HID, T]   (host-transposed)  -> moving operand of QKV matmuls
  qT/kT    [D, T]     per head           -> RoPE applied in this layout
  v        [T, D]     natural            -> PV stationary (via PE transpose)
  scoresT  [kt, qt]   exp'd on ACT, denominator via DVE adds + ones-matmul
  ctxT     [D, T]     -> stationary of the output projection

All matmuls run in fp32r (full PE speed); fp32r operands coming from DRAM
are rounded in-flight by casting SWDGE DMAs, on-device producers write fp32r
directly.
"""

import numpy as np

import concourse.bass as bass
import concourse.mybir as mybir
import concourse.tile as tile
from concourse import bacc, bass_utils
from concourse.bass import ts

F32 = mybir.dt.float32
F32R = mybir.dt.float32r
AF = mybir.ActivationFunctionType
OP = mybir.AluOpType

H, KV, D, HID, T = 16, 4, 128, 2048, 2048
THETA = 10000.0
N_CORES = 8
QH = H // N_CORES            # query heads per core = 2
TB = 512                     # token block (matmul moving N)
NTB = T // TB                # 4
HCN = HID // 128             # 16 h-chunks
NKT_TILES = T // 128         # 16 key tiles
SCALE = float(D) ** -0.5


def _build():
    nc = bacc.Bacc("TRN2", target_bir_lowering=False, debug=False,
                   num_devices=N_CORES)

    hT_d = nc.dram_tensor("hiddenT", [HID, T], F32, kind="ExternalInput").ap()
    w_d = nc.dram_tensor("w_local", [HID, 4 * 128], F32, kind="ExternalInput").ap()
    wo_d = nc.dram_tensor("wo_local", [2 * 128, HID], F32, kind="ExternalInput").ap()
    cos_d = nc.dram_tensor("cosT", [128, T], F32, kind="ExternalInput").ap()
    sin_d = nc.dram_tensor("sinT", [128, T], F32, kind="ExternalInput").ap()
    mask_d = nc.dram_tensor("masks", [128, 4 * TB], F32, kind="ExternalInput").ap()
    ones_d = nc.dram_tensor("ones", [128, 128], F32, kind="ExternalInput").ap()
    id_d = nc.dram_tensor("ident", [128, 128], F32, kind="ExternalInput").ap()
    out_d = nc.dram_tensor("out_partial", [T, HID], F32, kind="ExternalOutput").ap()

    with tile.TileContext(nc) as tc:
        with tc.tile_pool(name="const", bufs=1) as cpool, \
             tc.tile_pool(name="acts", bufs=1) as apool:
            # ---- resident constants ----
            w_sb = cpool.tile([128, HCN, 512], F32R)          # Wqkv slice
            nc.gpsimd.dma_start(w_sb[:], w_d.rearrange("(hc p) n -> p hc n", p=128))
            wo_sb = cpool.tile([128, 2, HID], F32R)           # Wo rows
            nc.gpsimd.dma_start(wo_sb[:], wo_d.rearrange("(c p) n -> p c n", p=128))
            cos_sb = cpool.tile([128, T], F32)
            nc.sync.dma_start(cos_sb[:], cos_d)
            sin_sb = cpool.tile([128, T], F32)
            nc.sync.dma_start(sin_sb[:], sin_d)
            mask_sb = cpool.tile([128, 4, TB], F32)
            nc.sync.dma_start(mask_sb[:], mask_d.rearrange("p (m n) -> p m n", n=TB))
            ones_sb = cpool.tile([128, 128], F32R)
            nc.gpsimd.dma_start(ones_sb[:], ones_d)
            id_sb = cpool.tile([128, 128], F32)
            nc.sync.dma_start(id_sb[:], id_d)

            # ---- persistent per-core activations ----
            qrT = [apool.tile([128, T], F32R, name=f"qrT{i}", tag=f"qrT{i}") for i in range(QH)]
            krT = apool.tile([128, T], F32R)
            v_nat = apool.tile([128, NKT_TILES, 128], F32R)
            ctxT = [apool.tile([128, T], F32R, name=f"ctxT{i}", tag=f"ctxT{i}") for i in range(QH)]

            hT_view = hT_d.rearrange("(hc p) t -> hc p t", p=128)

            # ================= Phase 1: QKV projection (+RoPE, v transpose) ==
            with tc.tile_pool(name="hstream", bufs=6) as hpool, \
                 tc.tile_pool(name="p1tmp", bufs=3) as tpool, \
                 tc.tile_pool(name="p1psum", bufs=1, space="PSUM") as qkv_ps_pool, \
                 tc.tile_pool(name="p1psumv", bufs=2, space="PSUM") as vps_pool:
                for b in range(NTB):
                    ps_qkv = [qkv_ps_pool.tile([128, TB], F32, name=f"psqkv{n}",
                                            tag=f"qkv{n}") for n in range(4)]
                    for hc in range(HCN):
                        hT_t = hpool.tile([128, TB], F32R)
                        nc.gpsimd.dma_start(hT_t[:], hT_view[hc, :, ts(b, TB)])
                        for n in range(4):
                            nc.tensor.matmul(ps_qkv[n][:], w_sb[:, hc, ts(n, 128)],
                                             hT_t[:], start=(hc == 0),
                                             stop=(hc == HCN - 1))
                    # q0, q1, k -> RoPE into qrT/krT; v -> transpose to natural
                    for n in range(3):
                        dst = qrT[n] if n < QH else krT
                        x_sb = tpool.tile([128, TB], F32, tag="ropex")
                        nc.scalar.copy(x_sb[:], ps_qkv[n][:])
                        xsw = tpool.tile([128, TB], F32, tag="ropesw")
                        nc.sync.dma_start(xsw[0:64, :], x_sb[64:128, :])
                        nc.sync.dma_start(xsw[64:128, :], x_sb[0:64, :])
                        t2 = tpool.tile([128, TB], F32, tag="ropet2")
                        nc.gpsimd.tensor_tensor(out=t2[:], in0=xsw[:],
                                                in1=sin_sb[:, ts(b, TB)], op=OP.mult)
                        m1 = tpool.tile([128, TB], F32, tag="ropem1")
                        nc.vector.tensor_tensor(out=m1[:], in0=x_sb[:],
                                                in1=cos_sb[:, ts(b, TB)], op=OP.mult)
                        nc.vector.tensor_tensor(out=dst[:, ts(b, TB)], in0=m1[:],
                                                in1=t2[:], op=OP.add)
                    vT_sb = tpool.tile([128, TB], F32, tag="vT")
                    nc.scalar.copy(vT_sb[:], ps_qkv[3][:])
                    for j in range(4):
                        ps_v = vps_pool.tile([128, 128], F32)
                        nc.tensor.transpose(ps_v[:], vT_sb[:, ts(j, 128)], id_sb[:])
                        nc.scalar.copy(v_nat[:, 4 * b + j, :], ps_v[:])

            # ================= Phase 2: causal attention =====================
            with tc.tile_pool(name="p2exp", bufs=4) as epool, \
                 tc.tile_pool(name="p2tmp", bufs=3) as t2pool, \
                 tc.tile_pool(name="p2ps_s", bufs=3, space="PSUM") as sps_pool, \
                 tc.tile_pool(name="p2ps_c", bufs=2, space="PSUM") as cps_pool, \
                 tc.tile_pool(name="p2ps_d", bufs=2, space="PSUM") as dps_pool:
                for qh in range(QH):
                    for b in range(NTB):
                        nkt = 4 * (b + 1)
                        ctx_ps = cps_pool.tile([128, TB], F32)
                        den = t2pool.tile([128, TB], F32R, tag="den")
                        for kt in range(nkt):
                            s_ps = sps_pool.tile([128, TB], F32)
                            nc.tensor.matmul(s_ps[:], krT[:, ts(kt, 128)],
                                             qrT[qh][:, ts(b, TB)],
                                             start=True, stop=True)
                            e_sb = epool.tile([128, TB], F32R, tag="exp")
                            nc.scalar.activation(e_sb[:], s_ps[:], AF.Exp,
                                                 scale=SCALE)
                            if kt >= 4 * b:
                                nc.vector.tensor_tensor(
                                    out=e_sb[:], in0=e_sb[:],
                                    in1=mask_sb[:, kt - 4 * b, :], op=OP.mult)
                            nc.tensor.matmul(ctx_ps[:], v_nat[:, kt, :], e_sb[:],
                                             start=(kt == 0), stop=(kt == nkt - 1))
                            if kt == 0:
                                nc.vector.tensor_copy(den[:], e_sb[:])
                            else:
                                nc.vector.tensor_tensor(out=den[:], in0=den[:],
                                                        in1=e_sb[:], op=OP.add)
                        den_ps = dps_pool.tile([128, TB], F32)
                        nc.tensor.matmul(den_ps[:], ones_sb[:], den[:],
                                         start=True, stop=True)
                        recip = t2pool.tile([128, TB], F32, tag="recip")
                        nc.vector.reciprocal(recip[:], den_ps[:])
                        nc.vector.tensor_tensor(out=ctxT[qh][:, ts(b, TB)],
                                                in0=ctx_ps[:], in1=recip[:],
                                                op=OP.mult)

            # ================= Phase 3: output projection ====================
            with tc.tile_pool(name="p3out", bufs=3) as opool, \
                 tc.tile_pool(name="p3psum", bufs=2, space="PSUM") as ops_pool:
                for tt in range(T // 128):
                    ps_o = [ops_pool.tile([128, 512], F32, name=f"pso{n}", tag=f"o{n}")
                            for n in range(4)]
                    for qh in range(QH):
                        for n in range(4):
                            nc.tensor.matmul(ps_o[n][:], ctxT[qh][:, ts(tt, 128)],
                                             wo_sb[:, qh, ts(n, 512)],
                                             start=(qh == 0), stop=(qh == QH - 1))
                    o_sb = opool.tile([128, HID], F32)
                    for n in range(4):
                        if n % 2 == 0:
                            nc.scalar.copy(o_sb[:, ts(n, 512)], ps_o[n][:])
                        else:
                            nc.vector.tensor_copy(o_sb[:, ts(n, 512)], ps_o[n][:])
                    nc.sync.dma_start(out_d[ts(tt, 128), :], o_sb[:])

    nc.compile()
    return nc


_NC_CACHE = None


def _get_nc():
    global _NC_CACHE
    if _NC_CACHE is None:
        _NC_CACHE = _build()
    return _NC_CACHE


def _host_tables(position_ids: np.ndarray):
    pos = np.asarray(position_ids, np.float32)
    inv_freq = (1.0 / (THETA ** (np.arange(0, D, 2, dtype=np.float32) / D)))
    ang = pos[:, None] * inv_freq[None, :]          # [T, 64] f32
    cos = np.cos(ang).T.astype(np.float32)          # [64, T]
    sin = np.sin(ang).T.astype(np.float32)
    cosT = np.concatenate([cos, cos], axis=0)       # [128, T]
    sinT = np.concatenate([-sin, sin], axis=0)
    return cosT, sinT


def _host_masks():
    r = np.arange(128)[:, None]
    c = np.arange(TB)[None, :]
    m = [(c - r - 128 * i >= 0).astype(np.float32) for i in range(4)]
    return np.concatenate(m, axis=1)                # [128, 4*TB]


def kernel(hidden_states, position_ids, Wqkv, Wo):
    hidden_states = np.asarray(hidden_states, np.float32)
    Wqkv = np.asarray(Wqkv, np.float32)
    Wo = np.asarray(Wo, np.float32)

    nc = _get_nc()

    hiddenT = np.ascontiguousarray(hidden_states.T)
    cosT, sinT = _host_tables(position_ids)
    masks = _host_masks()
    ones = np.ones((128, 128), np.float32)
    ident = np.eye(128, dtype=np.float32)

    wq = Wqkv[:, : H * D]
    wk = Wqkv[:, H * D: (H + KV) * D]
    wv = Wqkv[:, (H + KV) * D:]

    in_maps = []
    for c in range(N_CORES):
        kvh = (c * QH) // (H // KV)
        w_local = np.concatenate(
            [wq[:, (c * QH) * D: (c * QH + 1) * D],
             wq[:, (c * QH + 1) * D: (c * QH + 2) * D],
             wk[:, kvh * D: (kvh + 1) * D],
             wv[:, kvh * D: (kvh + 1) * D]], axis=1)
        wo_local = Wo[c * QH * D: (c + 1) * QH * D, :]
        in_maps.append({
            "hiddenT": hiddenT,
            "w_local": np.ascontiguousarray(w_local),
            "wo_local": np.ascontiguousarray(wo_local),
            "cosT": cosT, "sinT": sinT, "masks": masks,
            "ones": ones, "ident": ident,
        })

    res = bass_utils.run_bass_kernel_spmd(nc, in_maps,
                                          core_ids=list(range(N_CORES)))
    parts = np.stack([res.results[c]["out_partial"] for c in range(N_CORES)], 0)
    return parts.sum(axis=0, dtype=np.float32)


# revision 10
# speedup vs baseline: 1.2689x; 1.2689x over previous
"""Bass/Trainium2 kernel for BailingAttention (GQA prefill, causal, RoPE).

Sharding: tensor-parallel over heads across 8 NeuronCores. Each core computes
2 query heads + its group's shared KV head end-to-end (QKV projection, RoPE,
causal attention, output projection) and writes a partial [T, HID] output;
the host sums the 8 partials (the row-parallel all-reduce).

Layouts on device (partition dim first):
  hiddenT  [HID, T]   (host-transposed)  -> moving operand of QKV matmuls
  qT/kT    [D, T]     per head           -> RoPE applied in this layout
  v        [T, D]     natural            -> PV stationary (via PE transpose)
  scoresT  [kt, qt]   exp'd on ACT; denominator accumulated on PE via an
                      all-ones stationary (replicated column sums in PSUM)
  ctxT     [D, T]     -> stationary of the output projection

All matmuls run in fp32r (TF32-like: fp32 RNE-rounded to 11 mantissa bits)
at full PE speed. DRAM-sourced fp32r operands are pre-rounded bit-exactly on
the host so plain HWDGE DMAs suffice; on-device producers write fp32r
directly (the cast rounds).

The output projection for a 512-token block is emitted right after that
block's attention so its PSUM-evict copies and 1 MB output DMAs overlap the
next block's attention instead of running exposed at the end.
"""

import numpy as np

import concourse.bass as bass
import concourse.mybir as mybir
import concourse.tile as tile
from concourse import bacc, bass_utils
from concourse.bass import ts

F32 = mybir.dt.float32
F32R = mybir.dt.float32r
AF = mybir.ActivationFunctionType
OP = mybir.AluOpType

H, KV, D, HID, T = 16, 4, 128, 2048, 2048
THETA = 10000.0
N_CORES = 8
QH = H // N_CORES            # query heads per core = 2
TB = 512                     # token block (matmul moving N)
NTB = T // TB                # 4
HCN = HID // 128             # 16 h-chunks
NKT_TILES = T // 128         # 16 key tiles
SCALE = float(D) ** -0.5
PIPE = 2                     # attention software-pipeline depth (score MMs ahead)


def _to_f32r(a: np.ndarray) -> np.ndarray:
    """Round fp32 to fp32r bits (RNE to 11-bit mantissa) — bit-exactly what
    the hardware cast produces, so raw HWDGE DMA into f32r tiles is lossless."""
    b = np.ascontiguousarray(a, np.float32).view(np.uint32).astype(np.uint64)
    r = ((b + 0x7FF + ((b >> 12) & 1)) & 0xFFFFF000).astype(np.uint32)
    return r.view(np.float32)


def _build():
    nc = bacc.Bacc("TRN2", target_bir_lowering=False, debug=False,
                   num_devices=N_CORES)

    hT_d = nc.dram_tensor("hiddenT", [HID, T], F32R, kind="ExternalInput").ap()
    w_d = nc.dram_tensor("w_local", [HID, 4 * 128], F32R, kind="ExternalInput").ap()
    wo_d = nc.dram_tensor("wo_local", [2 * 128, HID], F32R, kind="ExternalInput").ap()
    cos_d = nc.dram_tensor("cosT", [128, T], F32, kind="ExternalInput").ap()
    sin_d = nc.dram_tensor("sinT", [128, T], F32, kind="ExternalInput").ap()
    mask_d = nc.dram_tensor("masks", [128, 4 * TB], F32, kind="ExternalInput").ap()
    ones_d = nc.dram_tensor("ones", [128, 128], F32R, kind="ExternalInput").ap()
    id_d = nc.dram_tensor("ident", [128, 128], F32, kind="ExternalInput").ap()
    out_d = nc.dram_tensor("out_partial", [T, HID], F32, kind="ExternalOutput").ap()

    with tile.TileContext(nc) as tc:
        with tc.tile_pool(name="const", bufs=1) as cpool, \
             tc.tile_pool(name="acts", bufs=1) as apool:
            # Resident constants. DMA emission order is load-bearing: the
            # QKV stream needs w-chunks + hT tiles first; everything else is
            # deferred so it doesn't delay the first matmuls.
            w_sb = cpool.tile([128, HCN, 512], F32R)
            wo_sb = cpool.tile([128, 2, HID], F32R)
            cos_sb = cpool.tile([128, T], F32)
            sin_sb = cpool.tile([128, T], F32)
            mask_sb = cpool.tile([128, 4, TB], F32)
            ones_sb = cpool.tile([128, 128], F32R)
            id_sb = cpool.tile([128, 128], F32)

            w_view = w_d.rearrange("(hc p) n -> hc p n", p=128)
            nc.scalar.dma_start(id_sb[:], id_d)
            nc.scalar.dma_start(ones_sb[:], ones_d)

            # persistent per-core activations
            qrT = [apool.tile([128, T], F32R, name=f"qrT{i}", tag=f"qrT{i}")
                   for i in range(QH)]
            krT = apool.tile([128, T], F32R)
            v_nat = apool.tile([128, NKT_TILES, 128], F32R)
            ctxT = [apool.tile([128, T], F32R, name=f"ctxT{i}", tag=f"ctxT{i}")
                    for i in range(QH)]

            hT_view = hT_d.rearrange("(hc p) t -> hc p t", p=128)

            # ================= Phase 1: QKV projection (+RoPE, v transpose) ==
            with tc.tile_pool(name="hstream", bufs=8) as hpool, \
                 tc.tile_pool(name="p1tmp", bufs=3) as tpool, \
                 tc.tile_pool(name="p1psum", bufs=1, space="PSUM") as qkv_ps_pool, \
                 tc.tile_pool(name="p1psumv", bufs=2, space="PSUM") as vps_pool:
                for b in range(NTB):
                    ps_qkv = [qkv_ps_pool.tile([128, TB], F32, name=f"psqkv{n}",
                                               tag=f"qkv{n}") for n in range(4)]
                    for hc in range(HCN):
                        if b == 0:
                            nc.sync.dma_start(w_sb[:, hc, :], w_view[hc])
                        hT_t = hpool.tile([128, TB], F32R)
                        nc.sync.dma_start(hT_t[:], hT_view[hc, :, ts(b, TB)])
                        for n in range(4):
                            nc.tensor.matmul(ps_qkv[n][:], w_sb[:, hc, ts(n, 128)],
                                             hT_t[:], start=(hc == 0),
                                             stop=(hc == HCN - 1))
                    if b == 0:
                        nc.scalar.dma_start(cos_sb[:], cos_d)
                        nc.scalar.dma_start(sin_sb[:], sin_d)
                    # q0, q1, k -> RoPE into qrT/krT; v -> transpose to natural
                    for n in range(3):
                        dst = qrT[n] if n < QH else krT
                        x_sb = tpool.tile([128, TB], F32, tag="ropex")
                        if n % 2 == 0:
                            nc.scalar.copy(x_sb[:], ps_qkv[n][:])
                        else:
                            nc.vector.tensor_copy(x_sb[:], ps_qkv[n][:])
                        xsw = tpool.tile([128, TB], F32, tag="ropesw")
                        nc.scalar.dma_start(xsw[0:64, :], x_sb[64:128, :])
                        nc.scalar.dma_start(xsw[64:128, :], x_sb[0:64, :])
                        t2 = tpool.tile([128, TB], F32, tag="ropet2")
                        nc.gpsimd.tensor_tensor(out=t2[:], in0=xsw[:],
                                                in1=sin_sb[:, ts(b, TB)], op=OP.mult)
                        m1 = tpool.tile([128, TB], F32, tag="ropem1")
                        nc.vector.tensor_tensor(out=m1[:], in0=x_sb[:],
                                                in1=cos_sb[:, ts(b, TB)], op=OP.mult)
                        nc.vector.tensor_tensor(out=dst[:, ts(b, TB)], in0=m1[:],
                                                in1=t2[:], op=OP.add)
                    vT_sb = tpool.tile([128, TB], F32, tag="vT")
                    nc.scalar.copy(vT_sb[:], ps_qkv[3][:])
                    for j in range(4):
                        ps_v = vps_pool.tile([128, 128], F32)
                        nc.tensor.transpose(ps_v[:], vT_sb[:, ts(j, 128)], id_sb[:])
                        nc.scalar.copy(v_nat[:, 4 * b + j, :], ps_v[:])
                    if b == 0:
                        nc.scalar.dma_start(
                            mask_sb[:], mask_d.rearrange("p (m n) -> p m n", n=TB))
                    if b == 1:
                        nc.scalar.dma_start(
                            wo_sb[:], wo_d.rearrange("(c p) n -> p c n", p=128))

            # ============ Phase 2+3: causal attention + output projection ====
            # Per 512-token block: attention for both heads, then that block's
            # output projection (its copies/DMAs overlap the next block).
            with tc.tile_pool(name="p2exp", bufs=PIPE + 3) as epool, \
                 tc.tile_pool(name="p2tmp", bufs=2) as t2pool, \
                 tc.tile_pool(name="p3out", bufs=2) as opool, \
                 tc.tile_pool(name="p2ps_s", bufs=PIPE + 1, space="PSUM") as sps_pool, \
                 tc.tile_pool(name="p2ps_c", bufs=1, space="PSUM") as cps_pool, \
                 tc.tile_pool(name="p2ps_d", bufs=1, space="PSUM") as dps_pool, \
                 tc.tile_pool(name="p3psum", bufs=3, space="PSUM") as ops_pool:
                for b in range(NTB):
                    nkt = 4 * (b + 1)
                    for qh in range(QH):
                        ctx_ps = cps_pool.tile([128, TB], F32, name="ctx_ps")
                        den_ps = dps_pool.tile([128, TB], F32, name="den_ps")
                        e_tiles = [None] * nkt

                        def emit_score(kt, b=b, qh=qh, e_tiles=e_tiles):
                            s_ps = sps_pool.tile([128, TB], F32, name="s_ps")
                            nc.tensor.matmul(s_ps[:], krT[:, ts(kt, 128)],
                                             qrT[qh][:, ts(b, TB)],
                                             start=True, stop=True)
                            e_sb = epool.tile([128, TB], F32R, name="e_sb",
                                              tag="exp")
                            nc.scalar.activation(e_sb[:], s_ps[:], AF.Exp,
                                                 scale=SCALE)
                            if kt >= 4 * b:   # diagonal tile: causal mask
                                nc.vector.tensor_tensor(
                                    out=e_sb[:], in0=e_sb[:],
                                    in1=mask_sb[:, kt - 4 * b, :], op=OP.mult)
                            e_tiles[kt] = e_sb

                        def emit_consume(kt, nkt=nkt, ctx_ps=ctx_ps,
                                         den_ps=den_ps, e_tiles=e_tiles):
                            e_sb = e_tiles[kt]
                            nc.tensor.matmul(ctx_ps[:], v_nat[:, kt, :], e_sb[:],
                                             start=(kt == 0), stop=(kt == nkt - 1))
                            nc.tensor.matmul(den_ps[:], ones_sb[:], e_sb[:],
                                             start=(kt == 0), stop=(kt == nkt - 1))

                        for kt in range(nkt + PIPE):
                            if kt < nkt:
                                emit_score(kt)
                            if kt >= PIPE:
                                emit_consume(kt - PIPE)

                        recip = t2pool.tile([128, TB], F32, tag="recip",
                                            name="recip")
                        nc.vector.reciprocal(recip[:], den_ps[:])
                        nc.vector.tensor_tensor(out=ctxT[qh][:, ts(b, TB)],
                                                in0=ctx_ps[:], in1=recip[:],
                                                op=OP.mult)

                    # ---- output projection for this block's 4 token tiles ----
                    for tt in range(4 * b, 4 * b + 4):
                        o_sb = opool.tile([128, HID], F32, name="o_sb")
                        for n in range(4):
                            ps_o = ops_pool.tile([128, 512], F32, name="ps_o")
                            for qh in range(QH):
                                nc.tensor.matmul(ps_o[:], ctxT[qh][:, ts(tt, 128)],
                                                 wo_sb[:, qh, ts(n, 512)],
                                                 start=(qh == 0),
                                                 stop=(qh == QH - 1))
                            if n % 2 == 0:
                                nc.scalar.copy(o_sb[:, ts(n, 512)], ps_o[:])
                            else:
                                nc.vector.tensor_copy(o_sb[:, ts(n, 512)], ps_o[:])
                        nc.sync.dma_start(out_d[ts(tt, 128), :], o_sb[:])

    nc.compile()
    return nc


_NC_CACHE = None


def _get_nc():
    global _NC_CACHE
    if _NC_CACHE is None:
        _NC_CACHE = _build()
    return _NC_CACHE


def _host_tables(position_ids: np.ndarray):
    pos = np.asarray(position_ids, np.float32)
    inv_freq = (1.0 / (THETA ** (np.arange(0, D, 2, dtype=np.float32) / D)))
    ang = pos[:, None] * inv_freq[None, :]          # [T, 64] f32
    cos = np.cos(ang).T.astype(np.float32)          # [64, T]
    sin = np.sin(ang).T.astype(np.float32)
    cosT = np.concatenate([cos, cos], axis=0)       # [128, T]
    sinT = np.concatenate([-sin, sin], axis=0)
    return cosT, sinT


def _host_masks():
    r = np.arange(128)[:, None]
    c = np.arange(TB)[None, :]
    m = [(c - r - 128 * i >= 0).astype(np.float32) for i in range(4)]
    return np.concatenate(m, axis=1)                # [128, 4*TB]


def kernel(hidden_states, position_ids, Wqkv, Wo):
    hidden_states = np.asarray(hidden_states, np.float32)
    Wqkv = np.asarray(Wqkv, np.float32)
    Wo = np.asarray(Wo, np.float32)

    nc = _get_nc()

    hiddenT = _to_f32r(hidden_states.T)
    cosT, sinT = _host_tables(position_ids)
    masks = _host_masks()
    ones = np.ones((128, 128), np.float32)
    ident = np.eye(128, dtype=np.float32)

    wq = Wqkv[:, : H * D]
    wk = Wqkv[:, H * D: (H + KV) * D]
    wv = Wqkv[:, (H + KV) * D:]

    in_maps = []
    for c in range(N_CORES):
        kvh = (c * QH) // (H // KV)
        w_local = np.concatenate(
            [wq[:, (c * QH) * D: (c * QH + 1) * D],
             wq[:, (c * QH + 1) * D: (c * QH + 2) * D],
             wk[:, kvh * D: (kvh + 1) * D],
             wv[:, kvh * D: (kvh + 1) * D]], axis=1)
        wo_local = Wo[c * QH * D: (c + 1) * QH * D, :]
        in_maps.append({
            "hiddenT": hiddenT,
            "w_local": _to_f32r(w_local),
            "wo_local": _to_f32r(wo_local),
            "cosT": cosT, "sinT": sinT, "masks": masks,
            "ones": ones, "ident": ident,
        })

    res = bass_utils.run_bass_kernel_spmd(nc, in_maps,
                                          core_ids=list(range(N_CORES)))
    parts = np.stack([res.results[c]["out_partial"] for c in range(N_CORES)], 0)
    return parts.sum(axis=0, dtype=np.float32)


# revision 13
# speedup vs baseline: 1.3056x; 1.0289x over previous
"""Bass/Trainium2 kernel for BailingAttention (GQA prefill, causal, RoPE).

Sharding: tensor-parallel over heads across 8 NeuronCores. Each core computes
2 query heads + its group's shared KV head end-to-end (QKV projection, RoPE,
causal attention, output projection) and writes a partial [T, HID] output;
the host sums the 8 partials (the row-parallel all-reduce).

Layouts on device (partition dim first):
  hiddenT  [HID, T]   (host-transposed)  -> moving operand of QKV matmuls
  qT/kT    [D, T]     per head           -> RoPE applied in this layout
  v        [T, D]     natural            -> PV stationary (via PE transpose)
  scoresT  [kt, qt]   exp'd on ACT; denominator accumulated on PE via an
                      all-ones stationary (replicated column sums in PSUM)
  ctxT     [D, T]     -> stationary of the output projection

All matmuls run in fp32r (TF32-like: fp32 RNE-rounded to 11 mantissa bits)
at full PE speed. DRAM-sourced fp32r operands are pre-rounded bit-exactly on
the host so plain HWDGE DMAs suffice; on-device producers write fp32r
directly (the cast rounds).

The output projection for a 512-token block is emitted right after that
block's attention so its PSUM-evict copies and 1 MB output DMAs overlap the
next block's attention instead of running exposed at the end.
"""

import numpy as np

import concourse.bass as bass
import concourse.mybir as mybir
import concourse.tile as tile
from concourse import bacc, bass_utils
from concourse.bass import ts

F32 = mybir.dt.float32
F32R = mybir.dt.float32r
AF = mybir.ActivationFunctionType
OP = mybir.AluOpType

H, KV, D, HID, T = 16, 4, 128, 2048, 2048
THETA = 10000.0
N_CORES = 8
QH = H // N_CORES            # query heads per core = 2
TB = 512                     # token block (matmul moving N)
NTB = T // TB                # 4
HCN = HID // 128             # 16 h-chunks
NKT_TILES = T // 128         # 16 key tiles
SCALE = float(D) ** -0.5
PIPE = 3                     # attention software-pipeline depth (score MMs ahead)


def _to_f32r(a: np.ndarray) -> np.ndarray:
    """Round fp32 to fp32r bits (RNE to 11-bit mantissa) — bit-exactly what
    the hardware cast produces, so raw HWDGE DMA into f32r tiles is lossless."""
    b = np.ascontiguousarray(a, np.float32).view(np.uint32).astype(np.uint64)
    r = ((b + 0x7FF + ((b >> 12) & 1)) & 0xFFFFF000).astype(np.uint32)
    return r.view(np.float32)


def _build():
    nc = bacc.Bacc("TRN2", target_bir_lowering=False, debug=False,
                   num_devices=N_CORES)

    hT_d = nc.dram_tensor("hiddenT", [HID, T], F32R, kind="ExternalInput").ap()
    w_d = nc.dram_tensor("w_local", [HID, 4 * 128], F32R, kind="ExternalInput").ap()
    wo_d = nc.dram_tensor("wo_local", [2 * 128, HID], F32R, kind="ExternalInput").ap()
    cos_d = nc.dram_tensor("cosT", [128, T], F32, kind="ExternalInput").ap()
    sin_d = nc.dram_tensor("sinT", [128, T], F32, kind="ExternalInput").ap()
    mask_d = nc.dram_tensor("masks", [128, 4 * TB], F32, kind="ExternalInput").ap()
    ones_d = nc.dram_tensor("ones", [128, 128], F32R, kind="ExternalInput").ap()
    id_d = nc.dram_tensor("ident", [128, 128], F32, kind="ExternalInput").ap()
    out_d = nc.dram_tensor("out_partial", [T, HID], F32, kind="ExternalOutput").ap()

    with tile.TileContext(nc) as tc:
        with tc.tile_pool(name="const", bufs=1) as cpool, \
             tc.tile_pool(name="acts", bufs=1) as apool:
            # Resident constants. DMA emission order is load-bearing: the
            # QKV stream needs w-chunks + hT tiles first; everything else is
            # deferred so it doesn't delay the first matmuls.
            w_sb = cpool.tile([128, HCN, 512], F32R)
            wo_sb = cpool.tile([128, 2, HID], F32R)
            cos_sb = cpool.tile([128, T], F32)
            sin_sb = cpool.tile([128, T], F32)
            mask_sb = cpool.tile([128, 4, TB], F32)
            ones_sb = cpool.tile([128, 128], F32R)
            id_sb = cpool.tile([128, 128], F32)

            w_view = w_d.rearrange("(hc p) n -> hc p n", p=128)

            # persistent per-core activations
            qrT = [apool.tile([128, T], F32R, name=f"qrT{i}", tag=f"qrT{i}")
                   for i in range(QH)]
            krT = apool.tile([128, T], F32R)
            v_nat = apool.tile([128, NKT_TILES, 128], F32R)
            ctxT = [apool.tile([128, T], F32R, name=f"ctxT{i}", tag=f"ctxT{i}")
                    for i in range(QH)]

            hT_view = hT_d.rearrange("(hc p) t -> hc p t", p=128)

            # ================= Phase 1: QKV projection (+RoPE, v transpose) ==
            with tc.tile_pool(name="hstream", bufs=8) as hpool, \
                 tc.tile_pool(name="p1tmp", bufs=3) as tpool, \
                 tc.tile_pool(name="p1psum", bufs=1, space="PSUM") as qkv_ps_pool, \
                 tc.tile_pool(name="p1psumv", bufs=2, space="PSUM") as vps_pool:
                for b in range(NTB):
                    ps_qkv = [qkv_ps_pool.tile([128, TB], F32, name=f"psqkv{n}",
                                               tag=f"qkv{n}") for n in range(4)]
                    for hc in range(HCN):
                        if b == 0:
                            nc.sync.dma_start(w_sb[:, hc, :], w_view[hc])
                        hT_t = hpool.tile([128, TB], F32R)
                        nc.sync.dma_start(hT_t[:], hT_view[hc, :, ts(b, TB)])
                        for n in range(4):
                            nc.tensor.matmul(ps_qkv[n][:], w_sb[:, hc, ts(n, 128)],
                                             hT_t[:], start=(hc == 0),
                                             stop=(hc == HCN - 1))
                    if b == 0:
                        nc.scalar.dma_start(id_sb[:], id_d)
                        nc.scalar.dma_start(ones_sb[:], ones_d)
                    nc.scalar.dma_start(cos_sb[:, ts(b, TB)], cos_d[:, ts(b, TB)])
                    nc.scalar.dma_start(sin_sb[:, ts(b, TB)], sin_d[:, ts(b, TB)])
                    # Evict all four accumulators first (frees PSUM for the
                    # next block's matmuls), then RoPE / v-transpose.
                    x_sbs = []
                    for n in range(4):
                        x_sb = tpool.tile([128, TB], F32, tag=f"ropex{n}",
                                          name=f"x_sb{n}")
                        if n % 2 == 0:
                            nc.scalar.copy(x_sb[:], ps_qkv[n][:])
                        else:
                            nc.vector.tensor_copy(x_sb[:], ps_qkv[n][:])
                        x_sbs.append(x_sb)
                    for n in range(3):
                        dst = qrT[n] if n < QH else krT
                        x_sb = x_sbs[n]
                        xsw = tpool.tile([128, TB], F32, tag="ropesw")
                        nc.scalar.dma_start(xsw[0:64, :], x_sb[64:128, :])
                        nc.scalar.dma_start(xsw[64:128, :], x_sb[0:64, :])
                        t2 = tpool.tile([128, TB], F32, tag="ropet2")
                        nc.gpsimd.tensor_tensor(out=t2[:], in0=xsw[:],
                                                in1=sin_sb[:, ts(b, TB)], op=OP.mult)
                        m1 = tpool.tile([128, TB], F32, tag="ropem1")
                        nc.gpsimd.tensor_tensor(out=m1[:], in0=x_sb[:],
                                                in1=cos_sb[:, ts(b, TB)], op=OP.mult)
                        nc.vector.tensor_tensor(out=dst[:, ts(b, TB)], in0=m1[:],
                                                in1=t2[:], op=OP.add)
                    vT_sb = x_sbs[3]
                    for j in range(4):
                        ps_v = vps_pool.tile([128, 128], F32)
                        nc.tensor.transpose(ps_v[:], vT_sb[:, ts(j, 128)], id_sb[:])
                        if j % 2 == 0:
                            nc.scalar.copy(v_nat[:, 4 * b + j, :], ps_v[:])
                        else:
                            nc.vector.tensor_copy(v_nat[:, 4 * b + j, :], ps_v[:])
                    if b == 2:
                        nc.scalar.dma_start(
                            mask_sb[:], mask_d.rearrange("p (m n) -> p m n", n=TB))
                        nc.scalar.dma_start(
                            wo_sb[:], wo_d.rearrange("(c p) n -> p c n", p=128))

            # ============ Phase 2+3: causal attention + output projection ====
            # Per 512-token block: attention for both heads, then that block's
            # output projection (its copies/DMAs overlap the next block).
            with tc.tile_pool(name="p2exp", bufs=PIPE + 3) as epool, \
                 tc.tile_pool(name="p2tmp", bufs=2) as t2pool, \
                 tc.tile_pool(name="p3out", bufs=2) as opool, \
                 tc.tile_pool(name="p2ps_s", bufs=PIPE + 1, space="PSUM") as sps_pool, \
                 tc.tile_pool(name="p2ps_c", bufs=1, space="PSUM") as cps_pool, \
                 tc.tile_pool(name="p2ps_d", bufs=1, space="PSUM") as dps_pool, \
                 tc.tile_pool(name="p3psum", bufs=2, space="PSUM") as ops_pool:
                for b in range(NTB):
                    nkt = 4 * (b + 1)
                    for qh in range(QH):
                        ctx_ps = cps_pool.tile([128, TB], F32, name="ctx_ps")
                        den_ps = dps_pool.tile([128, TB], F32, name="den_ps")
                        e_tiles = [None] * nkt

                        def emit_score(kt, b=b, qh=qh, e_tiles=e_tiles):
                            s_ps = sps_pool.tile([128, TB], F32, name="s_ps")
                            nc.tensor.matmul(s_ps[:], krT[:, ts(kt, 128)],
                                             qrT[qh][:, ts(b, TB)],
                                             start=True, stop=True)
                            e_sb = epool.tile([128, TB], F32R, name="e_sb",
                                              tag="exp")
                            nc.scalar.activation(e_sb[:], s_ps[:], AF.Exp,
                                                 scale=SCALE)
                            if kt >= 4 * b:   # diagonal tile: causal mask
                                nc.vector.tensor_tensor(
                                    out=e_sb[:], in0=e_sb[:],
                                    in1=mask_sb[:, kt - 4 * b, :], op=OP.mult)
                            e_tiles[kt] = e_sb

                        def emit_consume(kt, nkt=nkt, ctx_ps=ctx_ps,
                                         den_ps=den_ps, e_tiles=e_tiles):
                            e_sb = e_tiles[kt]
                            nc.tensor.matmul(ctx_ps[:], v_nat[:, kt, :], e_sb[:],
                                             start=(kt == 0), stop=(kt == nkt - 1))
                            nc.tensor.matmul(den_ps[:], ones_sb[:], e_sb[:],
                                             start=(kt == 0), stop=(kt == nkt - 1))

                        for kt in range(nkt + PIPE):
                            if kt < nkt:
                                emit_score(kt)
                            if kt >= PIPE:
                                emit_consume(kt - PIPE)

                        recip = t2pool.tile([128, TB], F32, tag="recip",
                                            name="recip")
                        nc.vector.reciprocal(recip[:], den_ps[:])
                        nc.vector.tensor_tensor(out=ctxT[qh][:, ts(b, TB)],
                                                in0=ctx_ps[:], in1=recip[:],
                                                op=OP.mult)

                    # ---- output projection for this block's 4 token tiles ----
                    for tt in range(4 * b, 4 * b + 4):
                        o_sb = opool.tile([128, HID], F32, name="o_sb")
                        for n in range(4):
                            ps_o = ops_pool.tile([128, 512], F32, name="ps_o")
                            for qh in range(QH):
                                nc.tensor.matmul(ps_o[:], ctxT[qh][:, ts(tt, 128)],
                                                 wo_sb[:, qh, ts(n, 512)],
                                                 start=(qh == 0),
                                                 stop=(qh == QH - 1))
                            if n == 0:
                                nc.scalar.copy(o_sb[:, ts(n, 512)], ps_o[:])
                            else:
                                nc.vector.tensor_copy(o_sb[:, ts(n, 512)], ps_o[:])
                        nc.sync.dma_start(out_d[ts(tt, 128), :], o_sb[:])

    nc.compile()
    return nc


_NC_CACHE = None


def _get_nc():
    global _NC_CACHE
    if _NC_CACHE is None:
        _NC_CACHE = _build()
    return _NC_CACHE


def _host_tables(position_ids: np.ndarray):
    pos = np.asarray(position_ids, np.float32)
    inv_freq = (1.0 / (THETA ** (np.arange(0, D, 2, dtype=np.float32) / D)))
    ang = pos[:, None] * inv_freq[None, :]          # [T, 64] f32
    cos = np.cos(ang).T.astype(np.float32)          # [64, T]
    sin = np.sin(ang).T.astype(np.float32)
    cosT = np.concatenate([cos, cos], axis=0)       # [128, T]
    sinT = np.concatenate([-sin, sin], axis=0)
    return cosT, sinT


def _host_masks():
    r = np.arange(128)[:, None]
    c = np.arange(TB)[None, :]
    m = [(c - r - 128 * i >= 0).astype(np.float32) for i in range(4)]
    return np.concatenate(m, axis=1)                # [128, 4*TB]


def kernel(hidden_states, position_ids, Wqkv, Wo):
    hidden_states = np.asarray(hidden_states, np.float32)
    Wqkv = np.asarray(Wqkv, np.float32)
    Wo = np.asarray(Wo, np.float32)

    nc = _get_nc()

    hiddenT = _to_f32r(hidden_states.T)
    cosT, sinT = _host_tables(position_ids)
    masks = _host_masks()
    ones = np.ones((128, 128), np.float32)
    ident = np.eye(128, dtype=np.float32)

    wq = Wqkv[:, : H * D]
    wk = Wqkv[:, H * D: (H + KV) * D]
    wv = Wqkv[:, (H + KV) * D:]

    in_maps = []
    for c in range(N_CORES):
        kvh = (c * QH) // (H // KV)
        w_local = np.concatenate(
            [wq[:, (c * QH) * D: (c * QH + 1) * D],
             wq[:, (c * QH + 1) * D: (c * QH + 2) * D],
             wk[:, kvh * D: (kvh + 1) * D],
             wv[:, kvh * D: (kvh + 1) * D]], axis=1)
        wo_local = Wo[c * QH * D: (c + 1) * QH * D, :]
        in_maps.append({
            "hiddenT": hiddenT,
            "w_local": _to_f32r(w_local),
            "wo_local": _to_f32r(wo_local),
            "cosT": cosT, "sinT": sinT, "masks": masks,
            "ones": ones, "ident": ident,
        })

    res = bass_utils.run_bass_kernel_spmd(nc, in_maps,
                                          core_ids=list(range(N_CORES)))
    parts = np.stack([res.results[c]["out_partial"] for c in range(N_CORES)], 0)
    return parts.sum(axis=0, dtype=np.float32)


# revision 17
# speedup vs baseline: 1.3318x; 1.0200x over previous
"""Bass/Trainium2 kernel for BailingAttention (GQA prefill, causal, RoPE).

Sharding: tensor-parallel over heads across 8 NeuronCores. Each core computes
2 query heads + its group's shared KV head end-to-end (QKV projection, RoPE,
causal attention, output projection) and writes a partial [T, HID] output;
the host sums the 8 partials (the row-parallel all-reduce).

Layouts on device (partition dim first):
  hiddenT  [HID, T]   (host-transposed)  -> moving operand of QKV matmuls
  qT/kT    [D, T]     per head           -> RoPE applied in this layout
  v        [T, D]     natural            -> PV stationary (via PE transpose)
  scoresT  [kt, qt]   exp'd on ACT; denominator accumulated on PE via an
                      all-ones stationary (replicated column sums in PSUM)
  ctxT     [D, T]     -> stationary of the output projection

All matmuls run in fp32r (TF32-like: fp32 RNE-rounded to 11 mantissa bits)
at full PE speed. DRAM-sourced fp32r operands are pre-rounded bit-exactly on
the host so plain HWDGE DMAs suffice; on-device producers write fp32r
directly (the cast rounds).

The output projection for a 512-token block is emitted right after that
block's attention so its PSUM-evict copies and 1 MB output DMAs overlap the
next block's attention instead of running exposed at the end.
"""

import numpy as np

import concourse.bass as bass
import concourse.mybir as mybir
import concourse.tile as tile
from concourse import bacc, bass_utils
from concourse.bass import ts

F32 = mybir.dt.float32
F32R = mybir.dt.float32r
AF = mybir.ActivationFunctionType
OP = mybir.AluOpType

H, KV, D, HID, T = 16, 4, 128, 2048, 2048
THETA = 10000.0
N_CORES = 8
QH = H // N_CORES            # query heads per core = 2
TB = 512                     # token block (matmul moving N)
NTB = T // TB                # 4
HCN = HID // 128             # 16 h-chunks
NKT_TILES = T // 128         # 16 key tiles
SCALE = float(D) ** -0.5
PIPE = 3                     # attention software-pipeline depth (score MMs ahead)


def _to_f32r(a: np.ndarray) -> np.ndarray:
    """Round fp32 to fp32r bits (RNE to 11-bit mantissa) — bit-exactly what
    the hardware cast produces, so raw HWDGE DMA into f32r tiles is lossless."""
    b = np.ascontiguousarray(a, np.float32).view(np.uint32).astype(np.uint64)
    r = ((b + 0x7FF + ((b >> 12) & 1)) & 0xFFFFF000).astype(np.uint32)
    return r.view(np.float32)


def _build():
    nc = bacc.Bacc("TRN2", target_bir_lowering=False, debug=False,
                   num_devices=N_CORES)

    hT_d = nc.dram_tensor("hiddenT", [HID, T], F32R, kind="ExternalInput").ap()
    w_d = nc.dram_tensor("w_local", [HID, 4 * 128], F32R, kind="ExternalInput").ap()
    wo_d = nc.dram_tensor("wo_local", [2 * 128, HID], F32R, kind="ExternalInput").ap()
    cos_d = nc.dram_tensor("cosT", [128, T], F32, kind="ExternalInput").ap()
    sin_d = nc.dram_tensor("sinT", [128, T], F32, kind="ExternalInput").ap()
    mask_d = nc.dram_tensor("masks", [128, 4 * TB], F32, kind="ExternalInput").ap()
    ones_d = nc.dram_tensor("ones", [128, 128], F32R, kind="ExternalInput").ap()
    id_d = nc.dram_tensor("ident", [128, 128], F32, kind="ExternalInput").ap()
    out_d = nc.dram_tensor("out_partial", [T, HID], F32, kind="ExternalOutput").ap()

    with tile.TileContext(nc) as tc:
        with tc.tile_pool(name="const", bufs=1) as cpool, \
             tc.tile_pool(name="acts", bufs=1) as apool:
            # Resident constants. DMA emission order is load-bearing: the
            # QKV stream needs w-chunks + hT tiles first; everything else is
            # deferred so it doesn't delay the first matmuls.
            w_sb = cpool.tile([128, HCN, 512], F32R)
            wo_sb = cpool.tile([128, 2, HID], F32R)
            cos_sb = cpool.tile([128, T], F32)
            sin_sb = cpool.tile([128, T], F32)
            mask_sb = cpool.tile([128, 4, TB], F32)
            ones_sb = cpool.tile([128, 128], F32R)
            id_sb = cpool.tile([128, 128], F32)

            w_view = w_d.rearrange("(hc p) n -> hc p n", p=128)

            # persistent per-core activations
            qrT = [apool.tile([128, T], F32R, name=f"qrT{i}", tag=f"qrT{i}")
                   for i in range(QH)]
            krT = apool.tile([128, T], F32R)
            v_nat = apool.tile([128, NKT_TILES, 128], F32R)
            ctxT = [apool.tile([128, T], F32R, name=f"ctxT{i}", tag=f"ctxT{i}")
                    for i in range(QH)]

            hT_view = hT_d.rearrange("(hc p) t -> hc p t", p=128)

            # ================= Phase 1: QKV projection (+RoPE, v transpose) ==
            with tc.tile_pool(name="hstream", bufs=8) as hpool, \
                 tc.tile_pool(name="p1tmp", bufs=3) as tpool, \
                 tc.tile_pool(name="p1psum", bufs=1, space="PSUM") as qkv_ps_pool, \
                 tc.tile_pool(name="p1psumv", bufs=2, space="PSUM") as vps_pool:
                for b in range(NTB):
                    ps_qkv = [qkv_ps_pool.tile([128, TB], F32, name=f"psqkv{n}",
                                               tag=f"qkv{n}") for n in range(4)]
                    for hc in range(HCN):
                        if b == 0:
                            nc.sync.dma_start(w_sb[:, hc, :], w_view[hc])
                        hT_t = hpool.tile([128, TB], F32R)
                        nc.sync.dma_start(hT_t[:], hT_view[hc, :, ts(b, TB)])
                        for n in range(4):
                            nc.tensor.matmul(ps_qkv[n][:], w_sb[:, hc, ts(n, 128)],
                                             hT_t[:], start=(hc == 0),
                                             stop=(hc == HCN - 1))
                    if b == 0:
                        nc.scalar.dma_start(id_sb[:], id_d)
                        nc.scalar.dma_start(ones_sb[:], ones_d)
                    nc.scalar.dma_start(cos_sb[:, ts(b, TB)], cos_d[:, ts(b, TB)])
                    nc.scalar.dma_start(sin_sb[:, ts(b, TB)], sin_d[:, ts(b, TB)])
                    # Evict all four accumulators first (frees PSUM for the
                    # next block's matmuls), then RoPE / v-transpose.
                    x_sbs = []
                    for n in range(4):
                        x_sb = tpool.tile([128, TB], F32, tag=f"ropex{n}",
                                          name=f"x_sb{n}")
                        if n % 2 == 0:
                            nc.scalar.copy(x_sb[:], ps_qkv[n][:])
                        else:
                            nc.vector.tensor_copy(x_sb[:], ps_qkv[n][:])
                        x_sbs.append(x_sb)
                    for n in range(3):
                        dst = qrT[n] if n < QH else krT
                        x_sb = x_sbs[n]
                        xsw = tpool.tile([128, TB], F32, tag="ropesw")
                        nc.scalar.dma_start(xsw[0:64, :], x_sb[64:128, :])
                        nc.scalar.dma_start(xsw[64:128, :], x_sb[0:64, :])
                        t2 = tpool.tile([128, TB], F32, tag="ropet2")
                        nc.gpsimd.tensor_tensor(out=t2[:], in0=xsw[:],
                                                in1=sin_sb[:, ts(b, TB)], op=OP.mult)
                        m1 = tpool.tile([128, TB], F32, tag="ropem1")
                        nc.gpsimd.tensor_tensor(out=m1[:], in0=x_sb[:],
                                                in1=cos_sb[:, ts(b, TB)], op=OP.mult)
                        nc.vector.tensor_tensor(out=dst[:, ts(b, TB)], in0=m1[:],
                                                in1=t2[:], op=OP.add)
                    vT_sb = x_sbs[3]
                    for j in range(4):
                        ps_v = vps_pool.tile([128, 128], F32)
                        nc.tensor.transpose(ps_v[:], vT_sb[:, ts(j, 128)], id_sb[:])
                        nc.vector.tensor_copy(v_nat[:, 4 * b + j, :], ps_v[:])
                    if b == 2:
                        nc.scalar.dma_start(
                            mask_sb[:], mask_d.rearrange("p (m n) -> p m n", n=TB))
                        nc.scalar.dma_start(
                            wo_sb[:], wo_d.rearrange("(c p) n -> p c n", p=128))

            # ============ Phase 2+3: causal attention + output projection ====
            # Per 512-token block: attention for both heads, then that block's
            # output projection (its copies/DMAs overlap the next block).
            with tc.tile_pool(name="p2exp", bufs=PIPE + 3) as epool, \
                 tc.tile_pool(name="p2tmp", bufs=2) as t2pool, \
                 tc.tile_pool(name="p3out", bufs=2) as opool, \
                 tc.tile_pool(name="p2ps_s", bufs=PIPE + 1, space="PSUM") as sps_pool, \
                 tc.tile_pool(name="p2ps_c", bufs=1, space="PSUM") as cps_pool, \
                 tc.tile_pool(name="p2ps_d", bufs=1, space="PSUM") as dps_pool, \
                 tc.tile_pool(name="p3psum", bufs=2, space="PSUM") as ops_pool:
                for b in range(NTB):
                    nkt = 4 * (b + 1)
                    for qh in range(QH):
                        ctx_ps = cps_pool.tile([128, TB], F32, name="ctx_ps")
                        den_ps = dps_pool.tile([128, TB], F32, name="den_ps")
                        e_tiles = [None] * nkt

                        def emit_score(kt, b=b, qh=qh, e_tiles=e_tiles):
                            s_ps = sps_pool.tile([128, TB], F32, name="s_ps")
                            nc.tensor.matmul(s_ps[:], krT[:, ts(kt, 128)],
                                             qrT[qh][:, ts(b, TB)],
                                             start=True, stop=True)
                            e_sb = epool.tile([128, TB], F32R, name="e_sb",
                                              tag="exp")
                            nc.scalar.activation(e_sb[:], s_ps[:], AF.Exp,
                                                 scale=SCALE)
                            if kt >= 4 * b:   # diagonal tile: causal mask
                                nc.vector.tensor_tensor(
                                    out=e_sb[:], in0=e_sb[:],
                                    in1=mask_sb[:, kt - 4 * b, :], op=OP.mult)
                            e_tiles[kt] = e_sb

                        def emit_consume(kt, nkt=nkt, ctx_ps=ctx_ps,
                                         den_ps=den_ps, e_tiles=e_tiles):
                            e_sb = e_tiles[kt]
                            nc.tensor.matmul(ctx_ps[:], v_nat[:, kt, :], e_sb[:],
                                             start=(kt == 0), stop=(kt == nkt - 1))
                            nc.tensor.matmul(den_ps[:], ones_sb[:], e_sb[:],
                                             start=(kt == 0), stop=(kt == nkt - 1))

                        for kt in range(nkt + PIPE):
                            if kt < nkt:
                                emit_score(kt)
                            if kt >= PIPE:
                                emit_consume(kt - PIPE)

                        recip = t2pool.tile([128, TB], F32, tag="recip",
                                            name="recip")
                        nc.vector.reciprocal(recip[:], den_ps[:])
                        nc.vector.tensor_tensor(out=ctxT[qh][:, ts(b, TB)],
                                                in0=ctx_ps[:], in1=recip[:],
                                                op=OP.mult)

                    # ---- output projection for this block's 4 token tiles ----
                    for tt in range(4 * b, 4 * b + 4):
                        o_sb = opool.tile([128, HID], F32, name="o_sb")
                        for n in range(4):
                            ps_o = ops_pool.tile([128, 512], F32, name="ps_o")
                            for qh in range(QH):
                                nc.tensor.matmul(ps_o[:], ctxT[qh][:, ts(tt, 128)],
                                                 wo_sb[:, qh, ts(n, 512)],
                                                 start=(qh == 0),
                                                 stop=(qh == QH - 1))
                            if n == 0:
                                nc.scalar.copy(o_sb[:, ts(n, 512)], ps_o[:])
                            else:
                                nc.vector.tensor_copy(o_sb[:, ts(n, 512)], ps_o[:])
                        nc.sync.dma_start(out_d[ts(tt, 128), :], o_sb[:])

    nc.compile()
    return nc


_NC_CACHE = None


def _get_nc():
    global _NC_CACHE
    if _NC_CACHE is None:
        _NC_CACHE = _build()
    return _NC_CACHE


def _host_tables(position_ids: np.ndarray):
    pos = np.asarray(position_ids, np.float32)
    inv_freq = (1.0 / (THETA ** (np.arange(0, D, 2, dtype=np.float32) / D)))
    ang = pos[:, None] * inv_freq[None, :]          # [T, 64] f32
    cos = np.cos(ang).T.astype(np.float32)          # [64, T]
    sin = np.sin(ang).T.astype(np.float32)
    cosT = np.concatenate([cos, cos], axis=0)       # [128, T]
    sinT = np.concatenate([-sin, sin], axis=0)
    return cosT, sinT


def _host_masks():
    r = np.arange(128)[:, None]
    c = np.arange(TB)[None, :]
    m = [(c - r - 128 * i >= 0).astype(np.float32) for i in range(4)]
    return np.concatenate(m, axis=1)                # [128, 4*TB]


def kernel(hidden_states, position_ids, Wqkv, Wo):
    hidden_states = np.asarray(hidden_states, np.float32)
    Wqkv = np.asarray(Wqkv, np.float32)
    Wo = np.asarray(Wo, np.float32)

    nc = _get_nc()

    hiddenT = _to_f32r(hidden_states.T)
    cosT, sinT = _host_tables(position_ids)
    masks = _host_masks()
    ones = np.ones((128, 128), np.float32)
    ident = np.eye(128, dtype=np.float32)

    wq = Wqkv[:, : H * D]
    wk = Wqkv[:, H * D: (H + KV) * D]
    wv = Wqkv[:, (H + KV) * D:]

    in_maps = []
    for c in range(N_CORES):
        kvh = (c * QH) // (H // KV)
        w_local = np.concatenate(
            [wq[:, (c * QH) * D: (c * QH + 1) * D],
             wq[:, (c * QH + 1) * D: (c * QH + 2) * D],
             wk[:, kvh * D: (kvh + 1) * D],
             wv[:, kvh * D: (kvh + 1) * D]], axis=1)
        wo_local = Wo[c * QH * D: (c + 1) * QH * D, :]
        in_maps.append({
            "hiddenT": hiddenT,
            "w_local": _to_f32r(w_local),
            "wo_local": _to_f32r(wo_local),
            "cosT": cosT, "sinT": sinT, "masks": masks,
            "ones": ones, "ident": ident,
        })

    res = bass_utils.run_bass_kernel_spmd(nc, in_maps,
                                          core_ids=list(range(N_CORES)))
    parts = np.stack([res.results[c]["out_partial"] for c in range(N_CORES)], 0)
    return parts.sum(axis=0, dtype=np.float32)


# revision 20
# speedup vs baseline: 1.3349x; 1.0024x over previous
"""Bass/Trainium2 kernel for BailingAttention (GQA prefill, causal, RoPE).

Sharding: tensor-parallel over heads across 8 NeuronCores. Each core computes
2 query heads + its group's shared KV head end-to-end (QKV projection, RoPE,
causal attention, output projection) and writes a partial [T, HID] output;
the host sums the 8 partials (the row-parallel all-reduce).

Layouts on device (partition dim first):
  hiddenT  [HID, T]   (host-transposed)  -> moving operand of QKV matmuls
  qT/kT    [D, T]     per head           -> RoPE applied in this layout
  v        [T, D]     natural            -> PV stationary (via PE transpose)
  scoresT  [kt, qt]   exp'd on ACT; denominator accumulated on PE via an
                      all-ones stationary (replicated column sums in PSUM)
  ctxT     [D, T]     -> stationary of the output projection

All matmuls run in fp32r (TF32-like: fp32 RNE-rounded to 11 mantissa bits)
at full PE speed. DRAM-sourced fp32r operands are pre-rounded bit-exactly on
the host so plain HWDGE DMAs suffice; on-device producers write fp32r
directly (the cast rounds).

The output projection for a 512-token block is emitted right after that
block's attention so its PSUM-evict copies and 1 MB output DMAs overlap the
next block's attention instead of running exposed at the end.
"""

import numpy as np

import concourse.bass as bass
import concourse.mybir as mybir
import concourse.tile as tile
from concourse import bacc, bass_utils
from concourse.bass import ts

F32 = mybir.dt.float32
F32R = mybir.dt.float32r
AF = mybir.ActivationFunctionType
OP = mybir.AluOpType

H, KV, D, HID, T = 16, 4, 128, 2048, 2048
THETA = 10000.0
N_CORES = 8
QH = H // N_CORES            # query heads per core = 2
TB = 512                     # token block (matmul moving N)
NTB = T // TB                # 4
HCN = HID // 128             # 16 h-chunks
NKT_TILES = T // 128         # 16 key tiles
SCALE = float(D) ** -0.5
PIPE = 3                     # attention software-pipeline depth (score MMs ahead)


def _to_f32r(a: np.ndarray) -> np.ndarray:
    """Round fp32 to fp32r bits (RNE to 11-bit mantissa) — bit-exactly what
    the hardware cast produces, so raw HWDGE DMA into f32r tiles is lossless."""
    b = np.ascontiguousarray(a, np.float32).view(np.uint32).astype(np.uint64)
    r = ((b + 0x7FF + ((b >> 12) & 1)) & 0xFFFFF000).astype(np.uint32)
    return r.view(np.float32)


def _build():
    nc = bacc.Bacc("TRN2", target_bir_lowering=False, debug=False,
                   num_devices=N_CORES)

    hT_d = nc.dram_tensor("hiddenT", [HID, T], F32R, kind="ExternalInput").ap()
    w_d = nc.dram_tensor("w_local", [HID, 4 * 128], F32R, kind="ExternalInput").ap()
    wo_d = nc.dram_tensor("wo_local", [2 * 128, HID], F32R, kind="ExternalInput").ap()
    cos_d = nc.dram_tensor("cosT", [128, T], F32, kind="ExternalInput").ap()
    sin_d = nc.dram_tensor("sinT", [128, T], F32, kind="ExternalInput").ap()
    mask_d = nc.dram_tensor("masks", [128, 4 * TB], F32, kind="ExternalInput").ap()
    ones_d = nc.dram_tensor("ones", [128, 128], F32R, kind="ExternalInput").ap()
    id_d = nc.dram_tensor("ident", [128, 128], F32, kind="ExternalInput").ap()
    out_d = nc.dram_tensor("out_partial", [T, HID], F32, kind="ExternalOutput").ap()

    with tile.TileContext(nc) as tc:
        with tc.tile_pool(name="const", bufs=1) as cpool, \
             tc.tile_pool(name="acts", bufs=1) as apool:
            # Resident constants. DMA emission order is load-bearing: the
            # QKV stream needs w-chunks + hT tiles first; everything else is
            # deferred so it doesn't delay the first matmuls.
            w_sb = cpool.tile([128, HCN, 512], F32R)
            wo_sb = cpool.tile([128, 2, HID], F32R)
            cos_sb = cpool.tile([128, T], F32)
            sin_sb = cpool.tile([128, T], F32)
            mask_sb = cpool.tile([128, 4, TB], F32)
            ones_sb = cpool.tile([128, 128], F32R)
            id_sb = cpool.tile([128, 128], F32)

            w_view = w_d.rearrange("(hc p) n -> hc p n", p=128)

            # persistent per-core activations
            qrT = [apool.tile([128, T], F32R, name=f"qrT{i}", tag=f"qrT{i}")
                   for i in range(QH)]
            krT = apool.tile([128, T], F32R)
            v_nat = apool.tile([128, NKT_TILES, 128], F32R)
            ctxT = [apool.tile([128, T], F32R, name=f"ctxT{i}", tag=f"ctxT{i}")
                    for i in range(QH)]

            hT_view = hT_d.rearrange("(hc p) t -> hc p t", p=128)

            # ================= Phase 1: QKV projection (+RoPE, v transpose) ==
            with tc.tile_pool(name="hstream", bufs=8) as hpool, \
                 tc.tile_pool(name="p1tmp", bufs=3) as tpool, \
                 tc.tile_pool(name="p1psum", bufs=1, space="PSUM") as qkv_ps_pool, \
                 tc.tile_pool(name="p1psumv", bufs=2, space="PSUM") as vps_pool:
                for b in range(NTB):
                    ps_qkv = [qkv_ps_pool.tile([128, TB], F32, name=f"psqkv{n}",
                                               tag=f"qkv{n}") for n in range(4)]
                    for hc in range(HCN):
                        if b == 0:
                            nc.sync.dma_start(w_sb[:, hc, :], w_view[hc])
                        hT_t = hpool.tile([128, TB], F32R)
                        nc.sync.dma_start(hT_t[:], hT_view[hc, :, ts(b, TB)])
                        for n in range(4):
                            nc.tensor.matmul(ps_qkv[n][:], w_sb[:, hc, ts(n, 128)],
                                             hT_t[:], start=(hc == 0),
                                             stop=(hc == HCN - 1))
                    if b == 0:
                        nc.scalar.dma_start(id_sb[:], id_d)
                        nc.scalar.dma_start(ones_sb[:], ones_d)
                    nc.scalar.dma_start(cos_sb[:, ts(b, TB)], cos_d[:, ts(b, TB)])
                    nc.scalar.dma_start(sin_sb[:, ts(b, TB)], sin_d[:, ts(b, TB)])
                    # Evict all four accumulators first (frees PSUM for the
                    # next block's matmuls), then RoPE / v-transpose.
                    x_sbs = []
                    for n in range(4):
                        x_sb = tpool.tile([128, TB], F32, tag=f"ropex{n}",
                                          name=f"x_sb{n}")
                        if n % 2 == 0:
                            nc.scalar.copy(x_sb[:], ps_qkv[n][:])
                        else:
                            nc.vector.tensor_copy(x_sb[:], ps_qkv[n][:])
                        x_sbs.append(x_sb)
                    for n in range(3):
                        dst = qrT[n] if n < QH else krT
                        x_sb = x_sbs[n]
                        xsw = tpool.tile([128, TB], F32, tag="ropesw")
                        nc.scalar.dma_start(xsw[0:64, :], x_sb[64:128, :])
                        nc.scalar.dma_start(xsw[64:128, :], x_sb[0:64, :])
                        t2 = tpool.tile([128, TB], F32, tag="ropet2")
                        nc.gpsimd.tensor_tensor(out=t2[:], in0=xsw[:],
                                                in1=sin_sb[:, ts(b, TB)], op=OP.mult)
                        m1 = tpool.tile([128, TB], F32, tag="ropem1")
                        nc.gpsimd.tensor_tensor(out=m1[:], in0=x_sb[:],
                                                in1=cos_sb[:, ts(b, TB)], op=OP.mult)
                        nc.vector.tensor_tensor(out=dst[:, ts(b, TB)], in0=m1[:],
                                                in1=t2[:], op=OP.add)
                    vT_sb = x_sbs[3]
                    for j in range(4):
                        ps_v = vps_pool.tile([128, 128], F32)
                        nc.tensor.transpose(ps_v[:], vT_sb[:, ts(j, 128)], id_sb[:])
                        nc.vector.tensor_copy(v_nat[:, 4 * b + j, :], ps_v[:])
                    if b == 2:
                        nc.scalar.dma_start(
                            mask_sb[:], mask_d.rearrange("p (m n) -> p m n", n=TB))
                        nc.scalar.dma_start(
                            wo_sb[:], wo_d.rearrange("(c p) n -> p c n", p=128))

            # ============ Phase 2+3: causal attention + output projection ====
            # Per 512-token block: attention for both heads, then that block's
            # output projection (its copies/DMAs overlap the next block).
            with tc.tile_pool(name="p2exp", bufs=PIPE + 5) as epool, \
                 tc.tile_pool(name="p2tmp", bufs=2) as t2pool, \
                 tc.tile_pool(name="p3out", bufs=2) as opool, \
                 tc.tile_pool(name="p2ps_s", bufs=PIPE + 1, space="PSUM") as sps_pool, \
                 tc.tile_pool(name="p2ps_c", bufs=1, space="PSUM") as cps_pool, \
                 tc.tile_pool(name="p2ps_d", bufs=1, space="PSUM") as dps_pool, \
                 tc.tile_pool(name="p3psum", bufs=2, space="PSUM") as ops_pool:
                for b in range(NTB):
                    nkt = 4 * (b + 1)
                    for qh in range(QH):
                        ctx_ps = cps_pool.tile([128, TB], F32, name="ctx_ps")
                        den_ps = dps_pool.tile([128, TB], F32, name="den_ps")
                        e_tiles = [None] * nkt

                        def emit_score(kt, b=b, qh=qh, e_tiles=e_tiles):
                            s_ps = sps_pool.tile([128, TB], F32, name="s_ps")
                            nc.tensor.matmul(s_ps[:], krT[:, ts(kt, 128)],
                                             qrT[qh][:, ts(b, TB)],
                                             start=True, stop=True)
                            e_sb = epool.tile([128, TB], F32R, name="e_sb",
                                              tag="exp")
                            nc.scalar.activation(e_sb[:], s_ps[:], AF.Exp,
                                                 scale=SCALE)
                            if kt >= 4 * b:   # diagonal tile: causal mask
                                nc.vector.tensor_tensor(
                                    out=e_sb[:], in0=e_sb[:],
                                    in1=mask_sb[:, kt - 4 * b, :], op=OP.mult)
                            e_tiles[kt] = e_sb

                        def emit_consume(kt, nkt=nkt, ctx_ps=ctx_ps,
                                         den_ps=den_ps, e_tiles=e_tiles):
                            e_sb = e_tiles[kt]
                            nc.tensor.matmul(ctx_ps[:], v_nat[:, kt, :], e_sb[:],
                                             start=(kt == 0), stop=(kt == nkt - 1))
                            nc.tensor.matmul(den_ps[:], ones_sb[:], e_sb[:],
                                             start=(kt == 0), stop=(kt == nkt - 1))

                        # Head 1's consumes start deeper so its score MMs
                        # cover head 0's recip/normalize chain (the single
                        # ctx PSUM bank frees only after that chain).
                        depth = PIPE if qh == 0 else min(PIPE + 2, nkt)
                        for kt in range(nkt + depth):
                            if kt < nkt:
                                emit_score(kt)
                            if kt >= depth:
                                emit_consume(kt - depth)

                        recip = t2pool.tile([128, TB], F32, tag="recip",
                                            name="recip")
                        nc.vector.reciprocal(recip[:], den_ps[:])
                        nc.vector.tensor_tensor(out=ctxT[qh][:, ts(b, TB)],
                                                in0=ctx_ps[:], in1=recip[:],
                                                op=OP.mult)

                    # ---- output projection for this block's 4 token tiles ----
                    for tt in range(4 * b, 4 * b + 4):
                        o_sb = opool.tile([128, HID], F32, name="o_sb")
                        for n in range(4):
                            ps_o = ops_pool.tile([128, 512], F32, name="ps_o")
                            for qh in range(QH):
                                nc.tensor.matmul(ps_o[:], ctxT[qh][:, ts(tt, 128)],
                                                 wo_sb[:, qh, ts(n, 512)],
                                                 start=(qh == 0),
                                                 stop=(qh == QH - 1))
                            if n == 0:
                                nc.scalar.copy(o_sb[:, ts(n, 512)], ps_o[:])
                            else:
                                nc.vector.tensor_copy(o_sb[:, ts(n, 512)], ps_o[:])
                        nc.sync.dma_start(out_d[ts(tt, 128), :], o_sb[:])

    nc.compile()
    return nc


_NC_CACHE = None


def _get_nc():
    global _NC_CACHE
    if _NC_CACHE is None:
        _NC_CACHE = _build()
    return _NC_CACHE


def _host_tables(position_ids: np.ndarray):
    pos = np.asarray(position_ids, np.float32)
    inv_freq = (1.0 / (THETA ** (np.arange(0, D, 2, dtype=np.float32) / D)))
    ang = pos[:, None] * inv_freq[None, :]          # [T, 64] f32
    cos = np.cos(ang).T.astype(np.float32)          # [64, T]
    sin = np.sin(ang).T.astype(np.float32)
    cosT = np.concatenate([cos, cos], axis=0)       # [128, T]
    sinT = np.concatenate([-sin, sin], axis=0)
    return cosT, sinT


def _host_masks():
    r = np.arange(128)[:, None]
    c = np.arange(TB)[None, :]
    m = [(c - r - 128 * i >= 0).astype(np.float32) for i in range(4)]
    return np.concatenate(m, axis=1)                # [128, 4*TB]


def kernel(hidden_states, position_ids, Wqkv, Wo):
    hidden_states = np.asarray(hidden_states, np.float32)
    Wqkv = np.asarray(Wqkv, np.float32)
    Wo = np.asarray(Wo, np.float32)

    nc = _get_nc()

    hiddenT = _to_f32r(hidden_states.T)
    cosT, sinT = _host_tables(position_ids)
    masks = _host_masks()
    ones = np.ones((128, 128), np.float32)
    ident = np.eye(128, dtype=np.float32)

    wq = Wqkv[:, : H * D]
    wk = Wqkv[:, H * D: (H + KV) * D]
    wv = Wqkv[:, (H + KV) * D:]

    in_maps = []
    for c in range(N_CORES):
        kvh = (c * QH) // (H // KV)
        w_local = np.concatenate(
            [wq[:, (c * QH) * D: (c * QH + 1) * D],
             wq[:, (c * QH + 1) * D: (c * QH + 2) * D],
             wk[:, kvh * D: (kvh + 1) * D],
             wv[:, kvh * D: (kvh + 1) * D]], axis=1)
        wo_local = Wo[c * QH * D: (c + 1) * QH * D, :]
        in_maps.append({
            "hiddenT": hiddenT,
            "w_local": _to_f32r(w_local),
            "wo_local": _to_f32r(wo_local),
            "cosT": cosT, "sinT": sinT, "masks": masks,
            "ones": ones, "ident": ident,
        })

    res = bass_utils.run_bass_kernel_spmd(nc, in_maps,
                                          core_ids=list(range(N_CORES)))
    parts = np.stack([res.results[c]["out_partial"] for c in range(N_CORES)], 0)
    return parts.sum(axis=0, dtype=np.float32)
